# revision 10
# baseline (speedup 1.0000x reference)
"""Trainium2 Bass kernel for nn_CustomizingAttention (B=16, QL=VL=512, HID=1024, NH=4, CH=10).

Sharding: data-parallel over batch across 8 NeuronCores (2 batches/core, no
collectives). Host-side prep: transpose query/value shards to feature-major,
fold conv1d weights into a [3, DIM] matrix (conv(prev) @ Wloc == shifted-prev
@ M3), fold 1/sqrt(DIM) into swq/sbq.

Device kernel (per core, fully unrolled Tile kernel):
  qT = Wq^T @ queryT (+bq)              feature-major [1024, 1024]
  vT = Wv^T @ valueT + locT (+bias)     feature-major
  per (batch, head): qq^T/kk^T (feature-major), vv (natural, bf16),
  scores = qq^T.T slices @ kk^T, softmax (no max-sub; scores are tiny),
  attn -> DRAM fp32; attn -> bf16 -> DRAM scratch -> xbar-transposed load,
  ctx^T = vv.T @ attn^T, out = tanh([ctx; query] @ Wout + bout) natural.
All matmuls float32r except ctx (bf16 attn/vv).
"""

import numpy as np
from contextlib import ExitStack

import concourse.bass as bass
import concourse.tile as tile
from concourse import bacc, mybir, bass_utils

B, QL, VL, HID, NH, CH = 16, 512, 512, 1024, 4, 10
DIM = HID // NH  # 256
NCORES = 8
BL = B // NCORES  # 2 batches per core
T = BL * QL  # 1024 token rows per core

FP32 = mybir.dt.float32
F32R = mybir.dt.float32r
BF16 = mybir.dt.bfloat16
AF = mybir.ActivationFunctionType

_CACHE = {}


def _r(ap):
    return ap.bitcast(F32R)


def _build():
    """Build and finalize the per-core Bass module (identical on all cores)."""
    nc = bacc.Bacc("TRN2", target_bir_lowering=False, debug=False,
                   num_devices=NCORES)

    def din(name, shape, dt=F32R):
        return nc.dram_tensor(name, shape, dt, kind="ExternalInput")

    qT_d = din("qT", [HID, T])
    vT_d = din("vT", [HID, T])
    # 8 (batch,head) rows of 3-shifted prev_attn, packed 4-per-tile at
    # partition offsets 0/32/64/96 (so matmul tile_position is legal)
    prev3_d = din("prev3", [2, 99, VL])
    m3_d = din("m3", [99, DIM])  # M3 replicated at offsets 0/32/64/96
    wq_d = din("wq", [HID, HID])
    wv_d = din("wv", [HID, HID])
    swq_d = din("swq", [DIM, DIM])
    swk_d = din("swk", [DIM, DIM])
    swv_d = din("swv", [DIM, DIM])
    wout_d = din("wout", [2 * HID, HID])
    bq_d = din("bq_t", [128, 8], FP32)
    bv_d = din("bv_t", [128, 8], FP32)
    sbq_d = din("sbq_t", [128, 2], FP32)
    sbk_d = din("sbk_t", [128, 2], FP32)
    sbv_d = din("sbv_r", [1, DIM])
    bout_d = din("bout_r", [1, HID])
    ones_d = din("ones_r", [1, 128])

    out_d = nc.dram_tensor("out", [T, HID], FP32, kind="ExternalOutput")
    attn_d = nc.dram_tensor("attn", [BL * NH, QL, VL], FP32,
                            kind="ExternalOutput")

    with tile.TileContext(nc) as tc, ExitStack() as ctx:
        ep = ctx.enter_context
        p_qT = ep(tc.tile_pool(name="p_qT", bufs=8))    # queryT resident
        p_x = ep(tc.tile_pool(name="p_x", bufs=2))      # valueT stream
        p_w = ep(tc.tile_pool(name="p_w", bufs=3))      # weight stream
        p_qs = ep(tc.tile_pool(name="p_qs", bufs=8))    # q^T
        p_vs = ep(tc.tile_pool(name="p_vs", bufs=8))    # v^T
        p_ctx = ep(tc.tile_pool(name="p_ctx", bufs=8))  # ctx^T
        p_sw = ep(tc.tile_pool(name="p_sw", bufs=6))    # sw weights
        p_sm = ep(tc.tile_pool(name="p_sm", bufs=1))    # small consts
        p_qq = ep(tc.tile_pool(name="p_qq", bufs=6))    # qq^T/kk^T
        p_vv = ep(tc.tile_pool(name="p_vv", bufs=6))    # vv bf16
        p_at = ep(tc.tile_pool(name="p_at", bufs=4))    # attn fp32
        p_ab = ep(tc.tile_pool(name="p_ab", bufs=4))    # attn bf16
        p_aT = ep(tc.tile_pool(name="p_aT", bufs=6))    # attn^T bf16
        p_rs = ep(tc.tile_pool(name="p_rs", bufs=6))    # rowsum/rowinv
        p_out = ep(tc.tile_pool(name="p_out", bufs=3))  # out tiles
        p_ps = ep(tc.tile_pool(name="p_ps", bufs=8, space="PSUM"))
        p_dr = ep(tc.tile_pool(name="p_dr", bufs=3, space="DRAM"))

        # ---- load resident/small tensors ----
        qT_sb = []
        for k in range(8):
            t = p_qT.tile([128, T], F32R, tag="qT", name=f"qT{k}")
            nc.sync.dma_start(t[:], qT_d.ap()[k * 128:(k + 1) * 128, :])
            qT_sb.append(t)

        def small(dram, shape, tag, dt=F32R):
            t = p_sm.tile(shape, dt, tag=tag, name=tag)
            nc.sync.dma_start(t[:], dram.ap())
            return t

        m3_sb = small(m3_d, [99, DIM], "m3")
        bq_sb = small(bq_d, [128, 8], "bq", FP32)
        bv_sb = small(bv_d, [128, 8], "bv", FP32)
        sbq_sb = small(sbq_d, [128, 2], "sbq", FP32)
        sbk_sb = small(sbk_d, [128, 2], "sbk", FP32)
        sbv_sb = small(sbv_d, [1, DIM], "sbv")
        bout_sb = small(bout_d, [1, HID], "bout")
        prev_sb = []
        for ti in range(2):
            t = p_sm.tile([99, VL], F32R, tag=f"prev{ti}", name=f"prev{ti}")
            nc.sync.dma_start(t[:], prev3_d.ap()[ti])
            prev_sb.append(t)
        ones_sb = small(ones_d, [1, 128], "ones")

        sw_sb = {}
        for nm, dram in (("q", swq_d), ("k", swk_d), ("v", swv_d)):
            for k2 in range(2):
                t = p_sw.tile([128, DIM], F32R, tag="sw", name=f"sw_{nm}{k2}")
                nc.sync.dma_start(t[:], dram.ap()[k2 * 128:(k2 + 1) * 128, :])
                sw_sb[(nm, k2)] = t

        # ---- stage A/B: q^T = Wq^T @ queryT + bq ; v^T likewise + loc ----
        qs_sb = [p_qs.tile([128, T], F32R, tag="qs", name=f"qs{i}") for i in range(8)]
        vs_sb = [p_vs.tile([128, T], F32R, tag="vs", name=f"vs{i}") for i in range(8)]

        for jh in range(2):  # output-feature half (chunk of 8 psum banks)
            ps = {}
            for k in range(8):
                wt = p_w.tile([128, 512], F32R, tag="w", name="wt")
                nc.sync.dma_start(
                    wt[:], wq_d.ap()[k * 128:(k + 1) * 128,
                                     jh * 512:(jh + 1) * 512])
                for j4 in range(4):
                    for th in range(2):
                        g = j4 * 2 + th
                        if k == 0:
                            ps[g] = p_ps.tile([128, 512], FP32, tag="ps", name="psg")
                        nc.tensor.matmul(
                            ps[g][:],
                            _r(wt[:, j4 * 128:(j4 + 1) * 128]),
                            _r(qT_sb[k][:, th * 512:(th + 1) * 512]),
                            start=(k == 0), stop=(k == 7))
            for j4 in range(4):
                j = jh * 4 + j4
                for th in range(2):
                    nc.scalar.activation(
                        qs_sb[j][:, th * 512:(th + 1) * 512],
                        ps[j4 * 2 + th][:], AF.Identity,
                        bias=bq_sb[:, j:j + 1])

        for jh in range(2):
            ps = {}
            for k in range(8):
                xt = p_x.tile([128, T], F32R, tag="vx", name="xt")
                nc.sync.dma_start(xt[:], vT_d.ap()[k * 128:(k + 1) * 128, :])
                wt = p_w.tile([128, 512], F32R, tag="w", name="wt")
                nc.sync.dma_start(
                    wt[:], wv_d.ap()[k * 128:(k + 1) * 128,
                                     jh * 512:(jh + 1) * 512])
                for j4 in range(4):
                    for th in range(2):
                        g = j4 * 2 + th
                        if k == 0:
                            ps[g] = p_ps.tile([128, 512], FP32, tag="ps", name="psg")
                        nc.tensor.matmul(
                            ps[g][:],
                            _r(wt[:, j4 * 128:(j4 + 1) * 128]),
                            _r(xt[:, th * 512:(th + 1) * 512]),
                            start=(k == 0), stop=False)
            for j4 in range(4):
                j = jh * 4 + j4
                h, dh = j // 2, j % 2
                for th in range(2):
                    g = j4 * 2 + th
                    # location-energy term: shifted prev_attn @ M3 (rank-3)
                    r = th * NH + h
                    ti, ri = r // 4, r % 4
                    nc.tensor.matmul(
                        ps[g][:],
                        _r(m3_sb[32 * ri:32 * ri + 3,
                                 dh * 128:(dh + 1) * 128]),
                        _r(prev_sb[ti][32 * ri:32 * ri + 3, :]),
                        start=False, stop=True,
                        tile_position=(32 * ri, 0))
                    nc.scalar.activation(
                        vs_sb[j][:, th * 512:(th + 1) * 512],
                        ps[g][:], AF.Identity, bias=bv_sb[:, j:j + 1])

        # ---- stage C: per (batch, head) attention ----
        ctx_sb = [p_ctx.tile([128, T], F32R, tag="ctx", name=f"ctx{i}") for i in range(8)]

        for bl in range(BL):
            for h in range(NH):
                n = bl * NH + h
                # qq^T, kk^T: [DIM, QL] feature-major
                qqT, kkT = [], []
                for nm, dst, bias_sb in (("q", qqT, sbq_sb),
                                         ("k", kkT, sbk_sb)):
                    src = qs_sb if nm == "q" else vs_sb
                    for dt2 in range(2):
                        psq = p_ps.tile([128, 512], FP32, tag="ps", name="psq")
                        for k2 in range(2):
                            nc.tensor.matmul(
                                psq[:],
                                _r(sw_sb[(nm, k2)][:, dt2 * 128:(dt2 + 1) * 128]),
                                _r(src[h * 2 + k2][:, bl * 512:(bl + 1) * 512]),
                                start=(k2 == 0), stop=(k2 == 1))
                        tt_ = p_qq.tile([128, 512], F32R, tag="qq", name="qqt")
                        nc.scalar.activation(tt_[:], psq[:], AF.Identity,
                                             bias=bias_sb[:, dt2:dt2 + 1])
                        dst.append(tt_)

                # vv: [QL, DIM] natural layout, bf16
                vv = []
                for tk in range(4):
                    psv = p_ps.tile([128, DIM], FP32, tag="ps", name="psv")
                    for k2 in range(2):
                        nc.tensor.matmul(
                            psv[:],
                            _r(vs_sb[h * 2 + k2][:, bl * 512 + tk * 128:
                                                 bl * 512 + (tk + 1) * 128]),
                            _r(sw_sb[("v", k2)][:]),
                            start=(k2 == 0), stop=False)
                    nc.tensor.matmul(psv[:], _r(ones_sb[:]), _r(sbv_sb[:]),
                                     start=False, stop=True)
                    vt_ = p_vv.tile([128, DIM], BF16, tag="vv", name="vvt")
                    nc.scalar.copy(vt_[:], psv[:])
                    vv.append(vt_)

                # scores + softmax (no max subtraction: |scores| < ~1)
                ab_dram = p_dr.tile([QL, VL], BF16, tag="abd", name="abd")
                for tq in range(4):
                    pss = p_ps.tile([128, 512], FP32, tag="ps", name="pss")
                    for dt2 in range(2):
                        nc.tensor.matmul(
                            pss[:],
                            _r(qqT[dt2][:, tq * 128:(tq + 1) * 128]),
                            _r(kkT[dt2][:]),
                            start=(dt2 == 0), stop=(dt2 == 1))
                    at = p_at.tile([128, 512], FP32, tag="at", name="at")
                    rs = p_rs.tile([128, 1], FP32, tag="rs", name="rs")
                    nc.scalar.activation(at[:], pss[:], AF.Exp,
                                         accum_out=rs[:])
                    ri = p_rs.tile([128, 1], FP32, tag="ri", name="ri")
                    nc.vector.reciprocal(ri[:], rs[:])
                    nc.vector.tensor_scalar_mul(at[:], at[:], ri[:])
                    nc.sync.dma_start(
                        attn_d.ap()[n, tq * 128:(tq + 1) * 128, :], at[:])
                    ab = p_ab.tile([128, 512], BF16, tag="ab", name="ab")
                    nc.vector.tensor_copy(ab[:], at[:])
                    nc.sync.dma_start(ab_dram[tq * 128:(tq + 1) * 128, :],
                                      ab[:])

                # attn^T via xbar transpose (bf16), then ctx^T
                aT = []
                for tk in range(4):
                    t = p_aT.tile([128, 512], BF16, tag="aT", name="aTt")
                    nc.sync.dma_start_transpose(
                        t[:], ab_dram[:, tk * 128:(tk + 1) * 128])
                    aT.append(t)
                for dt2 in range(2):
                    psc = p_ps.tile([128, 512], FP32, tag="ps", name="psc")
                    for tk in range(4):
                        nc.tensor.matmul(
                            psc[:],
                            vv[tk][:, dt2 * 128:(dt2 + 1) * 128],
                            aT[tk][:],
                            start=(tk == 0), stop=(tk == 3))
                    nc.scalar.copy(
                        ctx_sb[h * 2 + dt2][:, bl * 512:(bl + 1) * 512],
                        psc[:])

        # ---- stage D: out = tanh([ctx; query] @ Wout + bout) ----
        for jh in range(2):
            ps = {}
            for kf in range(16):
                wt = p_w.tile([128, 512], F32R, tag="w", name="wt")
                nc.sync.dma_start(
                    wt[:], wout_d.ap()[kf * 128:(kf + 1) * 128,
                                       jh * 512:(jh + 1) * 512])
                src = ctx_sb[kf] if kf < 8 else qT_sb[kf - 8]
                for tt in range(8):
                    if kf == 0:
                        ps[tt] = p_ps.tile([128, 512], FP32, tag="ps", name="psd")
                    nc.tensor.matmul(
                        ps[tt][:],
                        _r(src[:, tt * 128:(tt + 1) * 128]),
                        _r(wt[:]),
                        start=(kf == 0), stop=False)
            for tt in range(8):
                nc.tensor.matmul(
                    ps[tt][:], _r(ones_sb[:]),
                    _r(bout_sb[:, jh * 512:(jh + 1) * 512]),
                    start=False, stop=True)
                ot = p_out.tile([128, 512], FP32, tag="ot", name="ot")
                nc.scalar.activation(ot[:], ps[tt][:], AF.Tanh)
                nc.sync.dma_start(
                    out_d.ap()[tt * 128:(tt + 1) * 128,
                               jh * 512:(jh + 1) * 512], ot[:])

    nc.finalize()
    return nc


def _get_nc():
    if "nc" not in _CACHE:
        _CACHE["nc"] = _build()
    return _CACHE["nc"]


def _prep_in_maps(inputs):
    g = {k: np.asarray(v, dtype=np.float32) for k, v in inputs.items()}
    query, value, prev = g["query"], g["value"], g["prev_attn"]

    # weight folding (host, negligible)
    m3 = np.einsum("ck,cd->kd", g["conv_w"][:, 0, :], g["Wloc"]).astype(np.float32)
    biasloc = (g["conv_b"] @ g["Wloc"]).astype(np.float32)  # [DIM]
    bias_v = (g["bias"] + np.tile(biasloc, NH)).astype(np.float32)
    scale = np.float32(1.0 / np.sqrt(DIM))
    swq_s = (g["swq"] * scale).astype(np.float32)
    sbq_s = (g["sbq"] * scale).astype(np.float32)

    m3_pack = np.zeros((99, DIM), np.float32)
    for ri in range(4):
        m3_pack[32 * ri:32 * ri + 3] = m3

    shared = {
        "m3": m3_pack,
        "wq": g["Wq"], "wv": g["Wv"],
        "swq": swq_s, "swk": g["swk"], "swv": g["swv"],
        "wout": g["Wout"],
        "bq_t": np.ascontiguousarray(g["bq"].reshape(8, 128).T),
        "bv_t": np.ascontiguousarray(bias_v.reshape(8, 128).T),
        "sbq_t": np.ascontiguousarray(sbq_s.reshape(2, 128).T),
        "sbk_t": np.ascontiguousarray(g["sbk"].reshape(2, 128).T),
        "sbv_r": np.ascontiguousarray(g["sbv"].reshape(1, DIM)),
        "bout_r": np.ascontiguousarray(g["bout"].reshape(1, HID)),
        "ones_r": np.ones((1, 128), np.float32),
    }

    in_maps = []
    for c in range(NCORES):
        b0 = c * BL
        qTc = np.ascontiguousarray(
            query[b0:b0 + BL].reshape(T, HID).T)
        vTc = np.ascontiguousarray(
            value[b0:b0 + BL].reshape(T, HID).T)
        prev3 = np.zeros((2, 99, VL), np.float32)
        for bl in range(BL):
            for h in range(NH):
                row = prev[(b0 + bl) * NH + h]
                r = bl * NH + h
                ti, ri = r // 4, r % 4
                prev3[ti, 32 * ri + 0, 1:] = row[:-1]
                prev3[ti, 32 * ri + 1, :] = row
                prev3[ti, 32 * ri + 2, :-1] = row[1:]
        in_maps.append({"qT": qTc, "vT": vTc, "prev3": prev3, **shared})
    return in_maps


def kernel(**inputs):
    nc = _get_nc()
    in_maps = _prep_in_maps(inputs)
    res = bass_utils.run_bass_kernel_spmd(nc, in_maps,
                                          core_ids=list(range(NCORES)))
    out_full = np.empty((B, QL, HID), np.float32)
    attn_full = np.empty((NH * B, QL, VL), np.float32)
    for c in range(NCORES):
        oc = res.results[c]["out"].reshape(BL, QL, HID)
        ac = res.results[c]["attn"]
        for bl in range(BL):
            out_full[c * BL + bl] = oc[bl]
            for h in range(NH):
                attn_full[h * B + c * BL + bl] = ac[bl * NH + h]
    return out_full, attn_full
